# revision 1
# baseline (speedup 1.0000x reference)
"""CharEmbeddingCNN Trainium2 kernel.

Reference computation (per word of L=20 chars):
    xe = emb[x]                       # [L, 256] -> treated as [256, L]
    y_k = conv1d_valid(xe, w_k) + b_k # k in (3,4,5), 256 -> 256 channels
    out = relu(max over all (k, t) of y_k[:, t]) * (len != 0)

Strategy (data-parallel over 8 NeuronCores, 1024 words each):
  - Host packs weights as bf16 lhsT tiles [i, (k,dk), o], emb as a bf16
    DRAM table, and a f32 identity for PE transposes.
  - On device, `dma_gather(transpose=True)` gathers embedding rows for
    32-word sub-blocks directly into [128 part = ch%128, sub, ch//128, pos]
    layout (sub-block-major so each gather's output is contiguous).
  - Conv = PSUM-accumulated matmuls over (i_chunk, dk) with shifted access
    patterns. Word groups are [n_subs x n_words] rectangles chosen so every
    matmul free dim is >= ~448 (the PE issues one matmul per ~182 ns
    regardless of N, so small-N matmuls waste throughput):
      * per sub: words 0..27 as one group  (N = 504/476/448)
      * residual words 28..31 across 7 subs (N = 504/476/448)
  - Segment max over t via strided DVE reduce_max into per-k accumulators,
    incremental bias+max-combine across k, PE transpose to [word, ch], and
    a fused relu*mask on ScalarE on the way out - all overlapped with the
    matmul stream.
"""

import numpy as np
import ml_dtypes
from contextlib import ExitStack

import concourse.bacc as bacc
import concourse.tile as tile
from concourse import mybir
from concourse.bass_utils import run_bass_kernel_spmd

F32 = mybir.dt.float32
BF16 = mybir.dt.bfloat16
I16 = mybir.dt.int16

B, S, L = 64, 128, 20
EMB = 256
KS = (3, 4, 5)
NCORES = 8
W = (B * S) // NCORES          # words per core (1024)
SUB = 32                       # words per gather (num_idxs = 640, %128 == 0)
RW = 28                        # words 0..RW-1 of each sub form one group
NKDK = sum(KS)                 # 12 packed (k, dk) weight slices
WARMUP_MM = 22                 # dummy matmuls to warm the PE clock gate and
                               # bridge until the first group's input DMAs land
# First S_OH subs are computed from host-built one-hot inputs against
# alphabet-contraction weights (W @ emb.T folded on host): the gpsimd
# library load (~20us) gates the first dma_gather, and the one-hot path
# lets the matmul stream start at ~5us instead.
S_OH = 4


def _kdk_off(ki, dk):
    return sum(KS[:ki]) + dk


def _resid_tiles(nsub):
    """Residual groups covering words RW..31 of all subs: the one-hot subs
    first, then gathered subs in chunks of <= 7 (7 * 4 * 18 = 504 <= 512)."""
    out = [(0, min(S_OH, nsub))]
    s0 = min(S_OH, nsub)
    while s0 < nsub:
        a = min(7, nsub - s0)
        out.append((s0, a))
        s0 += a
    return out


def build_bass(words=W):
    assert words % SUB == 0
    nsub = words // SUB
    nwb = words // 128          # output word-blocks of 128
    resids = _resid_tiles(nsub)

    nc = bacc.Bacc(
        "TRN2",
        target_bir_lowering=False,
        debug=False,
        enable_asserts=False,
        num_swdge_queues=1,
    )

    n_oh = min(S_OH, words // SUB)
    xi_cols = words * L // 16
    xi_d = nc.dram_tensor("xi", [128, xi_cols], I16, kind="ExternalInput").ap()
    emb_d = nc.dram_tensor("emb", [EMB, EMB], BF16, kind="ExternalInput").ap()
    wpk_d = nc.dram_tensor("wpk", [EMB, NKDK, EMB], BF16, kind="ExternalInput").ap()
    wta_d = nc.dram_tensor("wta", [EMB, NKDK, EMB], BF16, kind="ExternalInput").ap()
    oh_d = nc.dram_tensor("oh", [128, n_oh * 2 * SUB * L], BF16,
                          kind="ExternalInput").ap()
    bias_d = nc.dram_tensor("bias", [128, 6], F32, kind="ExternalInput").ap()
    mask_d = nc.dram_tensor("maskp", [128, nwb], F32, kind="ExternalInput").ap()
    id_d = nc.dram_tensor("ident", [128, 128], F32, kind="ExternalInput").ap()
    out_d = nc.dram_tensor("out", [words, EMB], F32, kind="ExternalOutput").ap()

    with tile.TileContext(nc) as tc, ExitStack() as ctx:
        const_pool = ctx.enter_context(tc.tile_pool(name="const", bufs=1))
        xe_pool = ctx.enter_context(tc.tile_pool(name="xe", bufs=1))
        psum_pool = ctx.enter_context(tc.tile_pool(name="ps", bufs=2, space="PSUM"))
        psum_t_pool = ctx.enter_context(tc.tile_pool(name="pst", bufs=2, space="PSUM"))
        m_pool = ctx.enter_context(tc.tile_pool(name="m", bufs=1))
        tmp_pool = ctx.enter_context(tc.tile_pool(name="tmp", bufs=2))
        out_pool = ctx.enter_context(tc.tile_pool(name="outp", bufs=3))

        # Input DMAs ordered/sliced so the first matmul group's operands
        # (xi for gathers, k3 slices of wta, sub-0 one-hot) land first --
        # ~5.5MB of inputs take ~15us to drain, which otherwise gates the
        # first real matmul.
        xi_t = const_pool.tile([128, xi_cols], I16)
        wt = [const_pool.tile([128, NKDK, EMB], BF16, tag=f"wt{ic}",
                              name=f"wt{ic}") for ic in range(2)]
        wta = [const_pool.tile([128, NKDK, EMB], BF16, tag=f"wta{ic}",
                               name=f"wta{ic}") for ic in range(2)]
        oh_t = const_pool.tile([128, n_oh, 2, SUB * L], BF16)
        oh_v = oh_d[:].rearrange("p (s c j) -> p s c j", s=n_oh, c=2)
        for ic in range(2):
            nc.sync.dma_start(wta[ic][:, 0:3, :],
                              wta_d[ic * 128:(ic + 1) * 128, 0:3, :])
        nc.sync.dma_start(oh_t[:, 0, :, :], oh_v[:, 0, :, :])
        for ic in range(2):
            nc.sync.dma_start(wta[ic][:, 3:NKDK, :],
                              wta_d[ic * 128:(ic + 1) * 128, 3:NKDK, :])
        nc.sync.dma_start(xi_t[:], xi_d[:])
        for s in range(1, n_oh):
            nc.sync.dma_start(oh_t[:, s, :, :], oh_v[:, s, :, :])
        for ic in range(2):
            nc.sync.dma_start(wt[ic][:], wpk_d[ic * 128:(ic + 1) * 128, :, :])
        ident = const_pool.tile([128, 128], F32)
        nc.sync.dma_start(ident[:], id_d[:])
        bias_t = const_pool.tile([128, 6], F32)
        nc.sync.dma_start(bias_t[:], bias_d[:])
        mask_t = const_pool.tile([128, nwb], F32)
        nc.sync.dma_start(mask_t[:], mask_d[:])

        M = {}
        for ki in range(3):
            for oc in range(2):
                M[(ki, oc)] = m_pool.tile(
                    [128, words], F32, tag=f"m{ki}{oc}", name=f"m{ki}{oc}")
        C = [m_pool.tile([128, words], F32, tag=f"c{oc}", name=f"c{oc}")
             for oc in range(2)]

        # PE warm-up while the first gather's descriptor gen runs on GpSimd
        scratch = const_pool.tile([128, 512], BF16)
        nc.vector.memset(scratch[:], 0.0)
        warm = psum_pool.tile([128, 512], F32, tag="ps0")
        for _ in range(WARMUP_MM):
            nc.tensor.matmul(warm[:], scratch[:, :128], scratch[:],
                             start=True, stop=True)

        # embedding gathers for subs >= n_oh, up-front (block-major single
        # tile; GpSimd descriptor gen is serial, ~5.7us per 640-idx gather)
        xe = xe_pool.tile([128, nsub, 2, SUB * L], BF16)
        for s in range(n_oh, nsub):
            nc.gpsimd.dma_gather(
                xe[:, s, :, :], emb_d[:],
                xi_t[:, s * SUB * L // 16:(s + 1) * SUB * L // 16],
                SUB * L, SUB * L, EMB, transpose=True, single_packet=False,
                # all gathers on one queue: concurrent transpose-gathers on
                # different SWDGE queues interleave their xbar sprays and
                # corrupt the output (verified on HW).
                queue_num=0,
            )
        # [128, sub, word, t] views per channel chunk
        xv = [xe[:, :, ic, :].rearrange("p s (w t) -> p s w t", t=L)
              for ic in range(2)]
        ohv = [oh_t[:, :, ic, :].rearrange("p s (w t) -> p s w t", t=L)
               for ic in range(2)]

        def conv_group(rhs_fn, m_out_fn, lhs):
            """One [?,?] word-rectangle: 3 convs x 2 o_chunks, PSUM-
            accumulated over (i_chunk, dk), then segment-max into M."""
            for oc in range(2):
                for ki, k in enumerate(KS):
                    lk = L - k + 1
                    shape, rhss = rhs_fn(ki, lk)
                    ps = psum_pool.tile([128, *shape, lk], F32,
                                        tag=f"ps{ki}", name=f"ps{ki}")
                    n = 2 * k
                    i = 0
                    for ic in range(2):
                        for dk in range(k):
                            nc.tensor.matmul(
                                ps[:],
                                lhs[ic][:, _kdk_off(ki, dk),
                                        oc * 128:(oc + 1) * 128],
                                rhss(ic, dk),
                                start=(i == 0), stop=(i == n - 1),
                            )
                            i += 1
                    nc.vector.reduce_max(
                        m_out_fn(ki, oc), ps[:], axis=mybir.AxisListType.X)

        def sub_group(s):
            v, lhs, so = (ohv, wta, s) if s < n_oh else (xv, wt, s)
            def rhs_fn(ki, lk):
                return ((RW,),
                        lambda ic, dk: v[ic][:, so, 0:RW, dk:dk + lk])
            def m_out_fn(ki, oc):
                return M[(ki, oc)][:, s * SUB:s * SUB + RW]
            conv_group(rhs_fn, m_out_fn, lhs)

        def resid_group(s0, a):
            # [p, r, s, t] (r outer) -- fewer outer-dim AP steps than
            # [p, s, r, t]: each outer step costs ~9 PE cycles.
            v, lhs = (ohv, wta) if s0 < n_oh else (xv, wt)
            def rhs_fn(ki, lk):
                return ((SUB - RW, a),
                        lambda ic, dk: v[ic][:, s0:s0 + a, RW:SUB, dk:dk + lk]
                        .rearrange("p s w t -> p w s t"))
            def m_out_fn(ki, oc):
                return (M[(ki, oc)]
                        .rearrange("p (s r) -> p r s", r=SUB)
                        [:, RW:SUB, s0:s0 + a])
            conv_group(rhs_fn, m_out_fn, lhs)

        wb_done = 0
        covered = 0

        def combine(hi):
            """Fold M into C for columns [covered, hi)."""
            nonlocal covered
            sl = slice(covered, hi)
            n = hi - covered
            # bias adds on ScalarE, maxes on DVE: DVE tensor_scalar would
            # pick the 2-port perf mode, which locks the shared GpSimd SBUF
            # port and collides with gather descriptor generation (measured
            # 10x slowdown on both sides).
            for oc in range(2):
                t4 = tmp_pool.tile([128, n], F32, tag="t4", name="t4")
                nc.scalar.add(
                    C[oc][:, sl], M[(0, oc)][:, sl],
                    bias_t[:, 3 * oc:3 * oc + 1])
                nc.scalar.add(
                    t4[:], M[(1, oc)][:, sl], bias_t[:, 3 * oc + 1:3 * oc + 2])
                nc.vector.tensor_max(C[oc][:, sl], C[oc][:, sl], t4[:])
                nc.scalar.add(
                    t4[:], M[(2, oc)][:, sl], bias_t[:, 3 * oc + 2:3 * oc + 3])
                nc.vector.tensor_max(C[oc][:, sl], C[oc][:, sl], t4[:])
            covered = hi

        def emit_ready():
            """Emit finished 128-word output blocks. Called one sub-group
            after the combine was queued so the PE transpose doesn't
            head-of-line-block the matmul stream on the DVE combine."""
            nonlocal wb_done
            while (wb_done + 1) * 128 <= covered:
                wb = wb_done
                for oc in range(2):
                    pst = psum_t_pool.tile([128, 128], F32, tag="pst",
                                           name="pst")
                    nc.tensor.transpose(
                        pst[:], C[oc][:, wb * 128:(wb + 1) * 128], ident[:])
                    ot = out_pool.tile([128, 128], F32, tag="ot", name="ot")
                    nc.scalar.activation(
                        ot[:], pst[:], mybir.ActivationFunctionType.Relu,
                        scale=mask_t[:, wb:wb + 1])
                    nc.sync.dma_start(
                        out_d[wb * 128:(wb + 1) * 128,
                              oc * 128:(oc + 1) * 128], ot[:])
                wb_done += 1

        ri = 0
        for s in range(nsub):
            emit_ready()
            sub_group(s)
            if ri < len(resids) and s == resids[ri][0] + resids[ri][1] - 1:
                resid_group(*resids[ri])
                ri += 1
                combine((resids[ri - 1][0] + resids[ri - 1][1]) * SUB)
        emit_ready()
        assert ri == len(resids) and covered == words and wb_done == nwb

    nc.compile()
    return nc


def prep_shared(emb, w3, w4, w5, b3, b4, b5):
    emb_bf = np.ascontiguousarray(emb.astype(ml_dtypes.bfloat16))
    wpk = np.empty((EMB, NKDK, EMB), dtype=ml_dtypes.bfloat16)
    wta = np.empty((EMB, NKDK, EMB), dtype=ml_dtypes.bfloat16)
    emb64 = emb.astype(np.float64)
    for ki, w in enumerate((w3, w4, w5)):
        k = KS[ki]
        for dk in range(k):
            # wpk[i, off, o] = w[o, i, dk]
            wpk[:, _kdk_off(ki, dk), :] = w[:, :, dk].T.astype(ml_dtypes.bfloat16)
            # wta[c, off, o] = sum_i emb[c, i] w[o, i, dk]  (alphabet weights)
            wta[:, _kdk_off(ki, dk), :] = (
                emb64 @ w[:, :, dk].astype(np.float64).T
            ).astype(ml_dtypes.bfloat16)
    bias = np.empty((128, 6), dtype=np.float32)
    for oc in range(2):
        for ki, b in enumerate((b3, b4, b5)):
            bias[:, 3 * oc + ki] = b[oc * 128:(oc + 1) * 128]
    ident = np.eye(128, dtype=np.float32)
    return emb_bf, wpk, wta, bias, ident


def prep_core(xf, lensf, words=W):
    """Per-core index + mask packing. xf: [words, L] int32, lensf: [words]."""
    xi = xf.reshape(-1).astype(np.int16)               # words * L
    # dma_gather index layout: idx j -> partition j % 16, column j // 16,
    # replicated across the 8 gpsimd cores (16-partition stripes).
    cols = xi.reshape(-1, 16).T                        # [16, words*L/16]
    xi_t = np.ascontiguousarray(np.tile(cols, (8, 1)))  # [128, cols]
    nwb = words // 128
    maskp = (lensf.reshape(nwb, 128).T != 0).astype(np.float32)
    maskp = np.ascontiguousarray(maskp)                # [128, nwb]
    # one-hot encodings for the first S_OH subs: [128, n_oh*2*640],
    # oh[p, s, c, j] = (x[s*32 + j//20, j%20] == c*128 + p)
    n_oh = min(S_OH, words // SUB)
    pos = xf[:n_oh * SUB].reshape(-1)                  # n_oh * 640
    onehot = (pos[None, :] == np.arange(EMB)[:, None])
    oh = (onehot.reshape(2, 128, n_oh, SUB * L)
          .transpose(1, 2, 0, 3)                       # [128, s, c, j]
          .reshape(128, -1).astype(ml_dtypes.bfloat16))
    return xi_t, maskp, np.ascontiguousarray(oh)


_CACHE = {}


def _get_nc(words=W):
    if words not in _CACHE:
        _CACHE[words] = build_bass(words)
    return _CACHE[words]


def run(x, lens, emb, w3, b3, w4, b4, w5, b5, trace=False, **spmd_kwargs):
    x = np.asarray(x)
    lens = np.asarray(lens)
    emb = np.asarray(emb, dtype=np.float32)
    nc = _get_nc()
    emb_bf, wpk, wta, bias, ident = prep_shared(
        np.asarray(emb), np.asarray(w3), np.asarray(w4), np.asarray(w5),
        np.asarray(b3), np.asarray(b4), np.asarray(b5))
    xf = x.reshape(B * S, L)
    lensf = lens.reshape(B * S)
    in_maps = []
    for c in range(NCORES):
        sl = slice(c * W, (c + 1) * W)
        xi_t, maskp, oh = prep_core(xf[sl], lensf[sl])
        in_maps.append({
            "xi": xi_t, "emb": emb_bf, "wpk": wpk, "wta": wta, "bias": bias,
            "maskp": maskp, "ident": ident, "oh": oh,
        })
    res = run_bass_kernel_spmd(
        nc, in_maps, core_ids=list(range(NCORES)), trace=trace, **spmd_kwargs)
    out = np.concatenate([r["out"] for r in res.results], axis=0)
    return np.ascontiguousarray(out.reshape(B, S, EMB).astype(np.float32)), res


def kernel(x, lens, emb, w3, b3, w4, b4, w5, b5, **unused):
    out, _ = run(x, lens, emb, w3, b3, w4, b4, w5, b5)
    return out



# revision 2
# speedup vs baseline: 2.3101x; 2.3101x over previous
"""CharEmbeddingCNN Trainium2 kernel (fp8 DoubleRow, one-hot formulation).

Reference computation (per word of L=20 chars):
    xe = emb[x]                       # [L, 256] -> treated as [256, L]
    y_k = conv1d_valid(xe, w_k) + b_k # k in (3,4,5), 256 -> 256 channels
    out = relu(max over all (k, t) of y_k[:, t]) * (len != 0)

Strategy (data-parallel over 8 NeuronCores, 1024 words each):
  - Since xe columns are embedding rows, each conv tap is a table lookup:
    y_k[:, t] = sum_dk WE_k[dk][:, x[t+dk]] with WE_k[dk] = emb @ w_k[:,:,dk].T
    folded on host (float64) and quantized to fp8 e4m3 at a 2^8 scale.
    The rhs is then a one-hot encoding of the characters, which is EXACT in
    fp8, so the only numerical error is the single wta quantization
    (measured rel err ~1e-2 vs the 2e-2 budget).
  - The 256-deep contraction (one-hot over the alphabet) maps onto ONE
    fp8 matmul per (k, dk, oc) via perf_mode=DoubleRow (2 fp8 weights per
    PE cell): lhsT [128, 2, 128], rhs one-hot [128, 2, gw, lk], halving
    the matmul count vs bf16 and doubling the MAC rate.
  - Conv accumulates over dk in PSUM with shifted rhs windows. Word groups
    of 28 fill a PSUM bank (N = 504/476/448 f32).
  - Segment max over t via DVE reduce_max into per-k accumulators,
    incremental bias+max-combine (biases pre-scaled by 2^8), PE transpose
    to [word, ch], and a fused relu*(mask/2^8) on ScalarE on the way out,
    all overlapped with the matmul stream.
  - No gathers at all: the one-hot rhs (5.2 MB/core fp8) streams in from
    DRAM in word-chunks, trivially hidden behind compute.
"""

import numpy as np
import ml_dtypes
from contextlib import ExitStack

import concourse.bacc as bacc
import concourse.tile as tile
from concourse import mybir
from concourse.bass_utils import run_bass_kernel_spmd

F32 = mybir.dt.float32
FP8 = mybir.dt.float8e4
DR = mybir.MatmulPerfMode.DoubleRow

B, S, L = 64, 128, 20
EMB = 256
KS = (3, 4, 5)
NCORES = 8
W = (B * S) // NCORES          # words per core (1024)
NKDK = sum(KS)                 # 12 packed (k, dk) weight slices
GW = 28                        # words per matmul group (N = 504/476/448)
SCALE = 256.0                  # fp8 wta scale; undone by the output mask
WARMUP_MM = 16                 # dummy matmuls to warm the PE clock gate and
                               # bridge until the first group's input DMAs land
NP_FP8 = ml_dtypes.float8_e4m3  # TRN FP8_EXP4: bias 7, max normal 240


def _kdk_off(ki, dk):
    return sum(KS[:ki]) + dk


def build_bass(words=W):
    nwb = words // 128          # output word-blocks of 128
    nc = bacc.Bacc(
        "TRN2",
        target_bir_lowering=False,
        debug=False,
        enable_asserts=False,
        num_swdge_queues=1,
    )

    oh_d = nc.dram_tensor("oh", [128, 2 * words * L], FP8,
                          kind="ExternalInput").ap()
    wta_d = nc.dram_tensor("wta", [128, NKDK, 2, EMB], FP8,
                           kind="ExternalInput").ap()
    bias_d = nc.dram_tensor("bias", [128, 6], F32, kind="ExternalInput").ap()
    mask_d = nc.dram_tensor("maskp", [128, nwb], F32, kind="ExternalInput").ap()
    id_d = nc.dram_tensor("ident", [128, 128], F32, kind="ExternalInput").ap()
    out_d = nc.dram_tensor("out", [words, EMB], F32, kind="ExternalOutput").ap()

    groups = []
    w0 = 0
    while w0 < words:
        gw = min(GW, words - w0)
        groups.append((w0, gw))
        w0 += gw

    with tile.TileContext(nc) as tc, ExitStack() as ctx:
        const_pool = ctx.enter_context(tc.tile_pool(name="const", bufs=1))
        oh_pool = ctx.enter_context(tc.tile_pool(name="ohp", bufs=1))
        psum_pool = ctx.enter_context(tc.tile_pool(name="ps", bufs=2, space="PSUM"))
        psum_t_pool = ctx.enter_context(tc.tile_pool(name="pst", bufs=2, space="PSUM"))
        m_pool = ctx.enter_context(tc.tile_pool(name="m", bufs=1))
        tmp_pool = ctx.enter_context(tc.tile_pool(name="tmp", bufs=2))
        out_pool = ctx.enter_context(tc.tile_pool(name="outp", bufs=3))

        wta_t = const_pool.tile([128, NKDK, 2, EMB], FP8)
        oh_t = oh_pool.tile([128, 2, words, L], FP8)
        oh_v = oh_d[:].rearrange("p (c w t) -> p c w t", c=2, t=L)

        # Input DMAs ordered so the first group's operands land first.
        nc.sync.dma_start(wta_t[:, 0:3], wta_d[:, 0:3])
        nc.sync.dma_start(oh_t[:, :, 0:2 * GW, :], oh_v[:, :, 0:2 * GW, :])
        nc.sync.dma_start(wta_t[:, 3:NKDK], wta_d[:, 3:NKDK])
        ident = const_pool.tile([128, 128], F32)
        nc.sync.dma_start(ident[:], id_d[:])
        bias_t = const_pool.tile([128, 6], F32)
        nc.sync.dma_start(bias_t[:], bias_d[:])
        mask_t = const_pool.tile([128, nwb], F32)
        nc.sync.dma_start(mask_t[:], mask_d[:])
        CH = 4 * GW
        w0 = 2 * GW
        while w0 < words:
            w1 = min(w0 + CH, words)
            nc.sync.dma_start(oh_t[:, :, w0:w1, :], oh_v[:, :, w0:w1, :])
            w0 = w1

        M = {}
        for ki in range(3):
            for oc in range(2):
                M[(ki, oc)] = m_pool.tile(
                    [128, words], F32, tag=f"m{ki}{oc}", name=f"m{ki}{oc}")
        C = [m_pool.tile([128, words], F32, tag=f"c{oc}", name=f"c{oc}")
             for oc in range(2)]

        # PE warm-up on local scratch (no DMA dependency)
        scratch = const_pool.tile([128, 2, 256], FP8)
        nc.vector.memset(scratch[:], 0.0)
        warm = psum_pool.tile([128, 256], F32, tag="ps0")
        for _ in range(WARMUP_MM):
            nc.tensor.matmul(warm[:], scratch[:, :, 0:128], scratch[:],
                             start=True, stop=True, perf_mode=DR)

        wb_done = 0
        covered = 0

        def combine(hi):
            """Fold M into C for columns [covered, hi); bias is pre-scaled."""
            nonlocal covered
            sl = slice(covered, hi)
            n = hi - covered
            for oc in range(2):
                t4 = tmp_pool.tile([128, n], F32, tag="t4", name="t4")
                nc.scalar.add(
                    C[oc][:, sl], M[(0, oc)][:, sl],
                    bias_t[:, 3 * oc:3 * oc + 1])
                nc.scalar.add(
                    t4[:], M[(1, oc)][:, sl], bias_t[:, 3 * oc + 1:3 * oc + 2])
                nc.vector.tensor_max(C[oc][:, sl], C[oc][:, sl], t4[:])
                nc.scalar.add(
                    t4[:], M[(2, oc)][:, sl], bias_t[:, 3 * oc + 2:3 * oc + 3])
                nc.vector.tensor_max(C[oc][:, sl], C[oc][:, sl], t4[:])
            covered = hi

        def emit_ready():
            """Emit finished 128-word output blocks. Called one group after
            the combine was queued so the PE transpose doesn't head-of-line-
            block the matmul stream on the DVE combine."""
            nonlocal wb_done
            while (wb_done + 1) * 128 <= covered:
                wb = wb_done
                for oc in range(2):
                    pst = psum_t_pool.tile([128, 128], F32, tag="pst",
                                           name="pst")
                    nc.tensor.transpose(
                        pst[:], C[oc][:, wb * 128:(wb + 1) * 128], ident[:])
                    ot = out_pool.tile([128, 128], F32, tag="ot", name="ot")
                    nc.scalar.activation(
                        ot[:], pst[:], mybir.ActivationFunctionType.Relu,
                        scale=mask_t[:, wb:wb + 1])
                    nc.sync.dma_start(
                        out_d[wb * 128:(wb + 1) * 128,
                              oc * 128:(oc + 1) * 128], ot[:])
                wb_done += 1

        for (w0, gw) in groups:
            emit_ready()
            for oc in range(2):
                for ki, k in enumerate(KS):
                    lk = L - k + 1
                    ps = psum_pool.tile([128, gw, lk], F32,
                                        tag=f"ps{ki}", name=f"ps{ki}")
                    for dk in range(k):
                        nc.tensor.matmul(
                            ps[:],
                            wta_t[:, _kdk_off(ki, dk), :,
                                  oc * 128:(oc + 1) * 128],
                            oh_t[:, :, w0:w0 + gw, dk:dk + lk],
                            start=(dk == 0), stop=(dk == k - 1),
                            perf_mode=DR,
                        )
                    nc.vector.reduce_max(
                        M[(ki, oc)][:, w0:w0 + gw], ps[:],
                        axis=mybir.AxisListType.X)
            if w0 + gw - covered >= 128 or w0 + gw == words:
                combine(w0 + gw)
        emit_ready()
        assert covered == words and wb_done == nwb

    nc.compile()
    return nc


def prep_shared(emb, w3, w4, w5, b3, b4, b5):
    emb64 = np.asarray(emb, np.float64)
    wta = np.empty((128, NKDK, 2, EMB), dtype=NP_FP8)
    for ki, w in enumerate((w3, w4, w5)):
        k = KS[ki]
        w64 = np.asarray(w, np.float64)
        for dk in range(k):
            # t[a, o] = sum_i emb[a, i] w[o, i, dk], scaled into fp8 range
            t = (emb64 @ w64[:, :, dk].T) * SCALE
            wta[:, _kdk_off(ki, dk)] = (
                t.reshape(2, 128, EMB).transpose(1, 0, 2).astype(NP_FP8))
    bias = np.empty((128, 6), dtype=np.float32)
    for oc in range(2):
        for ki, b in enumerate((b3, b4, b5)):
            bias[:, 3 * oc + ki] = (
                np.asarray(b, np.float64)[oc * 128:(oc + 1) * 128] * SCALE)
    ident = np.eye(128, dtype=np.float32)
    return wta, bias, ident


def prep_core(xf, lensf, words=W):
    """Per-core one-hot + mask packing. xf: [words, L] int32, lensf: [words].
    oh[p, c*words*L + w*L + t] = (xf[w, t] == c*128 + p), fp8 exact."""
    pos = xf.reshape(-1)
    onehot = (np.arange(EMB, dtype=np.int32)[:, None] == pos[None, :])
    oh = (onehot.reshape(2, 128, -1).transpose(1, 0, 2)
          .reshape(128, -1).astype(NP_FP8))
    nwb = words // 128
    maskp = ((lensf.reshape(nwb, 128).T != 0).astype(np.float32)
             * np.float32(1.0 / SCALE))
    return np.ascontiguousarray(oh), np.ascontiguousarray(maskp)


_CACHE = {}


def _get_nc(words=W):
    if words not in _CACHE:
        _CACHE[words] = build_bass(words)
    return _CACHE[words]


def run(x, lens, emb, w3, b3, w4, b4, w5, b5, trace=False, **spmd_kwargs):
    x = np.asarray(x)
    lens = np.asarray(lens)
    nc = _get_nc()
    wta, bias, ident = prep_shared(
        np.asarray(emb), np.asarray(w3), np.asarray(w4), np.asarray(w5),
        np.asarray(b3), np.asarray(b4), np.asarray(b5))
    xf = x.reshape(B * S, L)
    lensf = lens.reshape(B * S)
    in_maps = []
    for c in range(NCORES):
        sl = slice(c * W, (c + 1) * W)
        oh, maskp = prep_core(xf[sl], lensf[sl])
        in_maps.append({
            "oh": oh, "wta": wta, "bias": bias, "maskp": maskp, "ident": ident,
        })
    res = run_bass_kernel_spmd(
        nc, in_maps, core_ids=list(range(NCORES)), trace=trace, **spmd_kwargs)
    out = np.concatenate([r["out"] for r in res.results], axis=0)
    return np.ascontiguousarray(out.reshape(B, S, EMB).astype(np.float32)), res


def kernel(x, lens, emb, w3, b3, w4, b4, w5, b5, **unused):
    out, _ = run(x, lens, emb, w3, b3, w4, b4, w5, b5)
    return out
